# revision 4
# baseline (speedup 1.0000x reference)
"""MoE (brute-force reference) kernel for 8 TRN2 NeuronCores.

Strategy: expert-parallel, 2 experts per core in two capacity slots.
Host routes tokens by gate_idx (top-k dedup), assigns the 8 largest
experts to slot 0 (capacity C0) and the 8 smallest to slot 1 (C1),
transposes so the device sees xt[slot] = X_e.T [D, C]. Each core runs
  hT[m] = gelu(sum_k w1T[k,m].T @ xT[k] + b1)   then
  yT[m] = sum_k w2T[k,m].T @ hT[k]
All matmul operands are fp16 (same PE rate as bf16); accumulation is
fp32 in PSUM. b1 is fused into the gelu activation; b2 and the
gate_score combine happen on host in exact fp32. Tokens beyond a
slot's capacity (a handful under the seed-0 routing) fall back to an
exact host path.

Startup is critical-path tuned: both HWDGE issue engines (sync +
scalar) interleave the first expert's xt/w1 chunk DMAs in consumption
order; a vector-memset-fed dummy-matmul warm-up keeps the PE HAM
activity window busy from the moment engines start so the clock gate
reaches 8/8 just as the real stream begins.
"""

import numpy as np

import concourse.bacc as bacc
import concourse.mybir as mybir
from concourse import tile
from concourse.bass_utils import run_bass_kernel_spmd

E, D, H, TOPK, T = 16, 1024, 2048, 2, 2048
NCORES = 8
EPC = E // NCORES          # experts (slots) per core
CAPS = (270, 238)          # slot capacities (seed-0: 8 largest experts
                           # max 297 -> 27 spill to host; 8 smallest max 238)
KD, KH, MD = D // 128, H // 128, D // 128  # 8, 16, 8

_F16 = np.float16
_CACHE: dict = {}


def _build(reps: int = 1):
    dt = mybir.dt.float16
    f32 = mybir.dt.float32
    nc = bacc.Bacc("TRN2", target_bir_lowering=False, debug=False,
                   num_devices=NCORES)
    xts = [nc.dram_tensor(f"xt{e}", [D, CAPS[e]], dt, kind="ExternalInput")
           for e in range(EPC)]
    w1t = nc.dram_tensor("w1t", [EPC, D, H], dt, kind="ExternalInput")
    w2t = nc.dram_tensor("w2t", [EPC, H, D], dt, kind="ExternalInput")
    b1 = nc.dram_tensor("b1", [EPC, 128, KH], f32, kind="ExternalInput")
    yts = [nc.dram_tensor(f"yt{e}", [D, CAPS[e]], dt, kind="ExternalOutput")
           for e in range(EPC)]

    gelu = mybir.ActivationFunctionType.Gelu_apprx_tanh
    MGRP = 8   # GEMM1 m-tiles per psum group (k-inner within a group)
    HH = H // 2

    with tile.TileContext(nc) as tc:
        with (
            tc.tile_pool(name="xtp", bufs=1) as xtp,
            tc.tile_pool(name="w1p", bufs=1) as w1p,
            tc.tile_pool(name="w2p", bufs=1) as w2p,
            tc.tile_pool(name="htp", bufs=1) as htp,
            tc.tile_pool(name="yp", bufs=1) as yp,
            tc.tile_pool(name="bp", bufs=1) as bp,
            tc.tile_pool(name="ps", bufs=1, space="PSUM") as psp,
        ):
            # ---- warm-up: PE activity from the instant engines start, so
            # the HAM clock gate ramps while the first DMAs stream in. The
            # zero tile is memset on the (otherwise idle) vector engine so
            # nothing gates the first LDWEIGHTS.
            zt = bp.tile([128, 128], dt, name="warmz", tag="warmz")
            nc.vector.memset(zt[:], 0.0)
            psw = psp.tile([128, 128], f32, name="psw", tag="ps7")
            for _ in range(20):
                nc.tensor.matmul(psw[:], zt[:], zt[:], start=True, stop=True)

            # ---- DMA plumbing -------------------------------------------
            # Two HWDGE issue chains (sync + scalar) interleave the first
            # expert's transfers in consumption order; gpsimd (SWDGE, slow
            # first-transfer latency) only carries mid-kernel y evictions.
            def w_dma(eng, pool, pfx, dram, e, k0, nk, col0, ncol):
                """One [128, nk, ncol] chunk of dram[e, k0*128:, col0:] ->
                tile; returns list of per-k [128, ncol] views."""
                tl = pool.tile([128, nk * ncol], dt, name=f"{pfx}_{e}_{k0}",
                               tag=f"{pfx}{e}_{k0}")
                eng.dma_start(
                    out=tl[:].rearrange("p (k m) -> p k m", k=nk),
                    in_=dram.ap()[e, k0 * 128:(k0 + nk) * 128,
                                  col0:col0 + ncol]
                        .rearrange("(k p) m -> p k m", p=128))
                return [tl[:, j * ncol:(j + 1) * ncol] for j in range(nk)]

            def xt_dma(eng, e, k0, nk):
                C = CAPS[e]
                tl = xtp.tile([128, nk * C], dt, name=f"xt_{e}_{k0}",
                              tag=f"xt{e}_{k0}")
                eng.dma_start(
                    out=tl[:].rearrange("p (k c) -> p k c", k=nk),
                    in_=xts[e].ap().rearrange("(k p) c -> p k c", p=128)
                        [:, k0:k0 + nk, :])
                return [tl[:, j * C:(j + 1) * C] for j in range(nk)]

            # --- expert/slot 0 startup: interleaved on both HWDGE chains in
            # consumption order (GEMM1 group 0 is k-outer over w1a slabs).
            # Scalar's issue chain must end by ~12.5us: the gelu table load
            # (~2.6us) and the first activations follow it in program order.
            xv0 = [None] * KD
            w1a0 = [None] * KD

            def xt0_dma(eng, k0, nk):
                xv0[k0:k0 + nk] = xt_dma(eng, 0, k0, nk)

            def w1a_dma(eng, k):
                w1a0[k:k + 1] = w_dma(eng, w1p, "w1a", w1t, 0, k, 1, 0, HH)

            xt0_dma(nc.sync, 0, 2)                     # sync 1: xt k0-1
            w1a_dma(nc.scalar, 0)                      # scal 1
            xt0_dma(nc.sync, 2, 2)                     # sync 2: xt k2-3
            w1a_dma(nc.scalar, 1)                      # scal 2
            w1a_dma(nc.sync, 2)                        # sync 3
            w1a_dma(nc.scalar, 3)                      # scal 3
            xt0_dma(nc.sync, 4, 2)                     # sync 4: xt k4-5
            w1a_dma(nc.scalar, 4)                      # scal 4
            w1a_dma(nc.sync, 5)                        # sync 5
            w1a_dma(nc.scalar, 6)                      # scal 5
            xt0_dma(nc.sync, 6, 2)                     # sync 6: xt k6-7
            w1a_dma(nc.scalar, 7)                      # scal 6

            # group-1 weights (w1 cols HH..H) as [2,2,2,2] slabs + b1, then
            # GEMM2 weights as [4,4] chunks, all ahead of their consumption.
            w1b0 = []
            w1b0 += w_dma(nc.scalar, w1p, "w1b", w1t, 0, 0, 2, HH, HH)
            w1b0 += w_dma(nc.scalar, w1p, "w1b", w1t, 0, 2, 2, HH, HH)
            b1s0 = bp.tile([128, KH], f32, name="b1s0", tag="b1s0")
            nc.sync.dma_start(out=b1s0[:], in_=b1.ap()[0])
            w1b0 += w_dma(nc.sync, w1p, "w1b", w1t, 0, 4, 2, HH, HH)
            w1b0 += w_dma(nc.sync, w1p, "w1b", w1t, 0, 6, 2, HH, HH)
            w2s0 = []
            for c in range(4):
                w2s0 += w_dma(nc.sync, w2p, "w2s", w2t, 0, 4 * c, 4, 0, D)

            # --- expert/slot 1 transfers: plenty of lead time, coarse
            # chunks, all on sync so scalar is free for the activations.
            xv1 = xt_dma(nc.sync, 1, 0, KD)
            w1a1, w1b1, w2s1 = [], [], []
            w1a1 += w_dma(nc.sync, w1p, "w1A", w1t, 1, 0, 4, 0, HH)
            w1a1 += w_dma(nc.sync, w1p, "w1A", w1t, 1, 4, 4, 0, HH)
            b1s1 = bp.tile([128, KH], f32, name="b1s1", tag="b1s1")
            nc.sync.dma_start(out=b1s1[:], in_=b1.ap()[1])
            w1b1 += w_dma(nc.sync, w1p, "w1B", w1t, 1, 0, 4, HH, HH)
            w1b1 += w_dma(nc.sync, w1p, "w1B", w1t, 1, 4, 4, HH, HH)
            for c in range(4):
                w2s1 += w_dma(nc.sync, w2p, "w2s", w2t, 1, 4 * c, 4, 0, D)

            # ---- compute ------------------------------------------------
            for e in range(EPC):
                C = CAPS[e]
                xv = xv0 if e == 0 else xv1
                w1h = [w1a0 if e == 0 else w1a1, w1b0 if e == 0 else w1b1]
                w2s = w2s0 if e == 0 else w2s1
                b1s = b1s0 if e == 0 else b1s1

                # GEMM1: hT[m] = gelu(sum_k w1[k][:,m].T @ xT[k] + b1[m])
                hts = [htp.tile([128, C], dt, name=f"ht{e}_{m}",
                                tag=f"ht{e}_{m}") for m in range(KH)]
                for g in range(2):
                    wsrc = w1h[g]
                    pss = [psp.tile([128, C], f32, name=f"ps1_{e}_{m}",
                                    tag=f"ps{m - g * MGRP}")
                           for m in range(g * MGRP, (g + 1) * MGRP)]
                    for k in range(KD):
                        for i in range(MGRP):
                            nc.tensor.matmul(
                                pss[i][:],
                                wsrc[k][:, i * 128:(i + 1) * 128],
                                xv[k],
                                start=(k == 0), stop=(k == KD - 1))
                    for i, m in enumerate(range(g * MGRP, (g + 1) * MGRP)):
                        nc.scalar.activation(
                            hts[m][:], pss[i][:], gelu,
                            bias=b1s[:, m:m + 1])

                # GEMM2: yT[m] = sum_k w2s[k][:,m].T @ hts[k]
                ytv = yts[e].ap().rearrange("(g p) c -> p g c", p=128)
                for m in range(MD):
                    ps = psp.tile([128, C], f32, name=f"ps2_{e}_{m}",
                                  tag=f"ps{m % MGRP}")
                    for k in range(KH):
                        nc.tensor.matmul(
                            ps[:],
                            w2s[k][:, m * 128:(m + 1) * 128],
                            hts[k][:],
                            start=(k == 0), stop=(k == KH - 1))
                    yo = yp.tile([128, C], dt, name=f"y{e}_{m}",
                                 tag=f"y{e}_{m}")
                    nc.vector.tensor_copy(out=yo[:], in_=ps[:])
                    # SWDGE keeps slot-0 y off the HWDGE weight chains; the
                    # last slot's y takes the (by then idle) sync HWDGE so
                    # the kernel tail isn't behind the SWDGE end-drain.
                    y_eng = nc.sync if e == EPC - 1 else nc.gpsimd
                    y_eng.dma_start(
                        out=ytv[:, m:m + 1, :],
                        in_=yo[:].rearrange("p (g c) -> p g c", g=1))
    nc.compile()
    return nc


def _get_nc(reps: int = 1):
    if reps not in _CACHE:
        _CACHE[reps] = _build(reps)
    return _CACHE[reps]


def _route(gate_idx, gate_score):
    """Dedup routing + slot assignment. Returns per-expert
    (tokens, weights, overflow_tokens, overflow_weights, core, slot)."""
    g = np.asarray(gate_idx).astype(np.int64)
    sc = np.asarray(gate_score, dtype=np.float32)
    toks_all, wts_all, counts = [], [], []
    for e in range(E):
        m0, m1 = g[:, 0] == e, g[:, 1] == e
        toks = np.flatnonzero(m0 | m1)
        toks_all.append(toks)
        wts_all.append((sc[:, 0] * m0 + sc[:, 1] * m1)[toks])
        counts.append(len(toks))
    order = np.argsort(-np.asarray(counts), kind="stable")
    out = [None] * E
    for rank, e in enumerate(order):
        slot = 0 if rank < NCORES else 1
        core = rank if rank < NCORES else rank - NCORES
        cap = CAPS[slot]
        toks, wts = toks_all[e], wts_all[e]
        out[e] = (toks[:cap], wts[:cap], toks[cap:], wts[cap:], core, slot)
    return out


def kernel(inp, gate_idx, gate_score, w1, b1, w2, b2):
    inp = np.asarray(inp, dtype=np.float32)
    gate_idx = np.asarray(gate_idx)
    gate_score = np.asarray(gate_score, dtype=np.float32)
    w1 = np.asarray(w1, dtype=np.float32)
    b1 = np.asarray(b1, dtype=np.float32)
    w2 = np.asarray(w2, dtype=np.float32)
    b2 = np.asarray(b2, dtype=np.float32)

    routes = _route(gate_idx, gate_score)

    w1t_all = np.ascontiguousarray(w1.transpose(0, 2, 1)).astype(_F16)
    w2t_all = np.ascontiguousarray(w2.transpose(0, 2, 1)).astype(_F16)
    b1r = np.ascontiguousarray(
        b1.reshape(E, KH, 128).transpose(0, 2, 1))  # [E, 128, KH]

    in_maps = [
        {"w1t": np.zeros((EPC, D, H), _F16),
         "w2t": np.zeros((EPC, H, D), _F16),
         "b1": np.zeros((EPC, 128, KH), np.float32),
         "xt0": np.zeros((D, CAPS[0]), _F16),
         "xt1": np.zeros((D, CAPS[1]), _F16)}
        for _ in range(NCORES)
    ]
    for e in range(E):
        toks, wts, otoks, owts, core, slot = routes[e]
        im = in_maps[core]
        im["w1t"][slot] = w1t_all[e]
        im["w2t"][slot] = w2t_all[e]
        im["b1"][slot] = b1r[e]
        if len(toks):
            im[f"xt{slot}"][:, :len(toks)] = inp[toks].T.astype(_F16)

    nc = _get_nc()
    res = run_bass_kernel_spmd(nc, in_maps, list(range(NCORES)))

    # Host combine: weight each expert's output columns by the (summed)
    # gate score and accumulate per token; add the b2 term (folded out of
    # the device kernel). Tokens are unique within an expert, so the
    # fancy-indexed += is safe.
    out = np.einsum("tk,tkd->td", gate_score,
                    b2[np.asarray(gate_idx).astype(np.int64)])
    out = np.ascontiguousarray(out, dtype=np.float32)
    for e in range(E):
        toks, wts, otoks, owts, core, slot = routes[e]
        if len(toks):
            y = res.results[core][f"yt{slot}"][:, :len(toks)].T
            out[toks] += wts[:, None] * y.astype(np.float32)
        if len(otoks):  # exact host fallback for capacity overflow
            hh = inp[otoks] @ w1[e].T + b1[e]
            hh = 0.5 * hh * (1.0 + np.tanh(
                np.sqrt(2.0 / np.pi) * (hh + 0.044715 * hh ** 3)))
            out[otoks] += owts[:, None] * (hh @ w2[e].T)
    return out


# revision 8
# speedup vs baseline: 1.0541x; 1.0541x over previous
"""MoE (brute-force reference) kernel for 8 TRN2 NeuronCores.

Strategy: expert-parallel, 2 experts per core in two capacity slots.
Host routes tokens by gate_idx (top-k dedup), assigns the 8 largest
experts to slot 0 (capacity C0=256) and the 8 smallest to slot 1
(C1=238), transposes so the device sees xt[slot] = X_e.T [D, C]. Each
core runs
  hT[m] = gelu(sum_k w1T[k,m].T @ xT[k] + b1)   then
  yT[m] = sum_k w2T[k,m].T @ hT[k]
All matmul operands are fp16 (same PE rate as bf16); accumulation is
fp32 in PSUM. b1 is fused into the gelu activation; b2 and the
gate_score combine happen on host in exact fp32. Tokens beyond a
slot's capacity (~2% under the seed-0 routing) fall back to an exact
host path.

Schedule notes (from NTFF traces):
- C<=256 keeps each PSUM chain within half a 2KB bank, giving 16
  logical banks: GEMM1 group0 -> ps0-7, group1 -> ps8-15, GEMM2 ->
  ps0-7 again. The scalar engine's gelu evictions (~368ns each) then
  never gate the PE at group boundaries.
- Both HWDGE issue chains are used: scalar gets exactly 6 w1a slabs
  (so its gelu table load + activations follow early), sync carries
  everything else in strict consumption order. DMA issue slots are
  ~0.65us and the global DMA-sem pool only allows ~11 outstanding
  transfers, so issue order == need order matters.
- A vector-memset-fed dummy-matmul warm-up keeps the PE HAM activity
  window busy from the instant engines start (~6.5us) so the 1.2GHz
  cold clock ramps to 2.4GHz just as the real stream begins.
- GEMM2 is k-outer (m-inner over 8 banks) for k-tiles 0..11 so weight
  chunks are consumed as they arrive, then a per-m tail (k12-15 +
  eviction) streams the y DMAs out before the kernel end.
"""

import numpy as np

import concourse.bacc as bacc
import concourse.mybir as mybir
from concourse import tile
from concourse.bass_utils import run_bass_kernel_spmd

E, D, H, TOPK, T = 16, 1024, 2048, 2, 2048
NCORES = 8
EPC = E // NCORES          # experts (slots) per core
CAPS = (256, 238)          # slot capacities; seed-0 overflow -> host
KD, KH, MD = D // 128, H // 128, D // 128  # 8, 16, 8

_F16 = np.float16
_CACHE: dict = {}


def _build(reps: int = 1):
    dt = mybir.dt.float16
    f32 = mybir.dt.float32
    nc = bacc.Bacc("TRN2", target_bir_lowering=False, debug=False,
                   num_devices=NCORES)
    xts = [nc.dram_tensor(f"xt{e}", [D, CAPS[e]], dt, kind="ExternalInput")
           for e in range(EPC)]
    w1t = nc.dram_tensor("w1t", [EPC, D, H], dt, kind="ExternalInput")
    w2t = nc.dram_tensor("w2t", [EPC, H, D], dt, kind="ExternalInput")
    b1 = nc.dram_tensor("b1", [EPC, 128, KH], f32, kind="ExternalInput")
    yts = [nc.dram_tensor(f"yt{e}", [D, CAPS[e]], dt, kind="ExternalOutput")
           for e in range(EPC)]

    gelu = mybir.ActivationFunctionType.Gelu_apprx_tanh
    MGRP = 8
    HH = H // 2

    with tile.TileContext(nc) as tc:
        with (
            tc.tile_pool(name="xtp", bufs=1) as xtp,
            tc.tile_pool(name="w1p", bufs=1) as w1p,
            tc.tile_pool(name="w2p", bufs=1) as w2p,
            tc.tile_pool(name="htp", bufs=1) as htp,
            tc.tile_pool(name="yp", bufs=1) as yp,
            tc.tile_pool(name="bp", bufs=1) as bp,
            tc.tile_pool(name="ps", bufs=1, space="PSUM") as psp,
        ):
            # PSUM: 8 physical banks, each [128, 512] f32, allocated once.
            # C<=256 keeps every accumulation chain within half a bank, so
            # column slices give 16 independent chains (subtile deps).
            banks = [psp.tile([128, 512], f32, name=f"bank{i}", tag=f"ps{i}")
                     for i in range(MGRP)]

            # ---- warm-up (see module docstring)
            zt = bp.tile([128, 128], dt, name="warmz", tag="warmz")
            nc.vector.memset(zt[:], 0.0)
            for _ in range(13):
                nc.tensor.matmul(banks[7][:, :128], zt[:], zt[:],
                                 start=True, stop=True)

            # ---- DMA plumbing
            def w_dma(eng, pool, pfx, dram, e, k0, nk, col0, ncol):
                tl = pool.tile([128, nk * ncol], dt, name=f"{pfx}_{e}_{k0}",
                               tag=f"{pfx}{e}_{k0}")
                eng.dma_start(
                    out=tl[:].rearrange("p (k m) -> p k m", k=nk),
                    in_=dram.ap()[e, k0 * 128:(k0 + nk) * 128,
                                  col0:col0 + ncol]
                        .rearrange("(k p) m -> p k m", p=128))
                return [tl[:, j * ncol:(j + 1) * ncol] for j in range(nk)]

            def xt_dma(eng, e, k0, nk):
                C = CAPS[e]
                tl = xtp.tile([128, nk * C], dt, name=f"xt_{e}_{k0}",
                              tag=f"xt{e}_{k0}")
                eng.dma_start(
                    out=tl[:].rearrange("p (k c) -> p k c", k=nk),
                    in_=xts[e].ap().rearrange("(k p) c -> p k c", p=128)
                        [:, k0:k0 + nk, :])
                return [tl[:, j * C:(j + 1) * C] for j in range(nk)]

            # --- slot 0 startup, interleaved on both chains in need order
            xv0 = [None] * KD
            w1a0 = [None] * KD

            def xt0_dma(eng, k0, nk):
                xv0[k0:k0 + nk] = xt_dma(eng, 0, k0, nk)

            def w1a_dma(eng, k):
                w1a0[k:k + 1] = w_dma(eng, w1p, "w1a", w1t, 0, k, 1, 0, HH)

            xt0_dma(nc.sync, 0, 2)
            w1a_dma(nc.scalar, 0)
            xt0_dma(nc.sync, 2, 2)
            w1a_dma(nc.scalar, 1)
            w1a_dma(nc.sync, 2)
            w1a_dma(nc.scalar, 3)
            xt0_dma(nc.sync, 4, 2)
            w1a_dma(nc.scalar, 4)
            w1a_dma(nc.sync, 5)
            w1a_dma(nc.scalar, 6)
            xt0_dma(nc.sync, 6, 2)
            w1a_dma(nc.scalar, 7)
            # scalar chain ends here: gelu table load + activations follow.

            b1s0 = bp.tile([128, KH], f32, name="b1s0", tag="b1s0")
            nc.sync.dma_start(out=b1s0[:], in_=b1.ap()[0])
            w1b0 = []
            for k0 in range(0, KD, 2):
                w1b0 += w_dma(nc.sync, w1p, "w1b", w1t, 0, k0, 2, HH, HH)
            w2s0 = []
            for c in range(4):
                w2s0 += w_dma(nc.sync, w2p, "w2s", w2t, 0, 4 * c, 4, 0, D)

            # --- slot 1 transfers: coarse chunks, all on sync
            xv1 = xt_dma(nc.sync, 1, 0, KD)
            w1a1 = (w_dma(nc.sync, w1p, "w1A", w1t, 1, 0, 4, 0, HH) +
                    w_dma(nc.sync, w1p, "w1A", w1t, 1, 4, 4, 0, HH))
            b1s1 = bp.tile([128, KH], f32, name="b1s1", tag="b1s1")
            nc.sync.dma_start(out=b1s1[:], in_=b1.ap()[1])
            w1b1 = (w_dma(nc.sync, w1p, "w1B", w1t, 1, 0, 4, HH, HH) +
                    w_dma(nc.sync, w1p, "w1B", w1t, 1, 4, 4, HH, HH))
            w2s1 = []
            for c in range(4):
                w2s1 += w_dma(nc.sync, w2p, "w2s", w2t, 1, 4 * c, 4, 0, D)

            # ---- compute
            for e in range(EPC):
                C = CAPS[e]
                xv = xv0 if e == 0 else xv1
                w1h = [w1a0 if e == 0 else w1a1, w1b0 if e == 0 else w1b1]
                w2s = w2s0 if e == 0 else w2s1
                b1s = b1s0 if e == 0 else b1s1

                # GEMM1: group 0 -> bank halves [0:C], group 1 -> [256:256+C];
                # k-outer so group0 is paced by w1a slab arrival at startup.
                hts = [htp.tile([128, C], dt, name=f"ht{e}_{m}",
                                tag=f"ht{e}_{m}") for m in range(KH)]
                for g in range(2):
                    wsrc = w1h[g]
                    off = 256 * g
                    pss = [banks[i][:, off:off + C] for i in range(MGRP)]
                    for k in range(KD):
                        for i in range(MGRP):
                            nc.tensor.matmul(
                                pss[i],
                                wsrc[k][:, i * 128:(i + 1) * 128],
                                xv[k],
                                start=(k == 0), stop=(k == KD - 1))
                    for i, m in enumerate(range(g * MGRP, (g + 1) * MGRP)):
                        nc.scalar.activation(
                            hts[m][:], pss[i], gelu,
                            bias=b1s[:, m:m + 1])

                # GEMM2 (bank halves [0:C], freed by group0's acts): k-outer
                # for k 0..11 (chunk-paced), then per-m tail (k12-15 +
                # eviction) so y DMAs stream before kernel end.
                ytv = yts[e].ap().rearrange("(g p) c -> p g c", p=128)
                pss = [banks[m][:, 0:C] for m in range(MD)]
                for k in range(12):
                    for m in range(MD):
                        nc.tensor.matmul(
                            pss[m],
                            w2s[k][:, m * 128:(m + 1) * 128],
                            hts[k][:],
                            start=(k == 0), stop=False)
                for m in range(MD):
                    for k in range(12, KH):
                        nc.tensor.matmul(
                            pss[m],
                            w2s[k][:, m * 128:(m + 1) * 128],
                            hts[k][:],
                            start=False, stop=(k == KH - 1))
                    yo = yp.tile([128, C], dt, name=f"y{e}_{m}",
                                 tag=f"y{e}_{m}")
                    nc.vector.tensor_copy(out=yo[:], in_=pss[m])
                    y_eng = nc.sync if e == EPC - 1 else nc.gpsimd
                    y_eng.dma_start(
                        out=ytv[:, m:m + 1, :],
                        in_=yo[:].rearrange("p (g c) -> p g c", g=1))
    nc.compile()
    return nc


def _get_nc(reps: int = 1):
    if reps not in _CACHE:
        _CACHE[reps] = _build(reps)
    return _CACHE[reps]


def _route(gate_idx, gate_score):
    """Dedup routing + slot assignment. Returns per-expert
    (tokens, weights, overflow_tokens, overflow_weights, core, slot)."""
    g = np.asarray(gate_idx).astype(np.int64)
    sc = np.asarray(gate_score, dtype=np.float32)
    toks_all, wts_all, counts = [], [], []
    for e in range(E):
        m0, m1 = g[:, 0] == e, g[:, 1] == e
        toks = np.flatnonzero(m0 | m1)
        toks_all.append(toks)
        wts_all.append((sc[:, 0] * m0 + sc[:, 1] * m1)[toks])
        counts.append(len(toks))
    order = np.argsort(-np.asarray(counts), kind="stable")
    out = [None] * E
    for rank, e in enumerate(order):
        slot = 0 if rank < NCORES else 1
        core = rank if rank < NCORES else rank - NCORES
        cap = CAPS[slot]
        toks, wts = toks_all[e], wts_all[e]
        out[e] = (toks[:cap], wts[:cap], toks[cap:], wts[cap:], core, slot)
    return out


def kernel(inp, gate_idx, gate_score, w1, b1, w2, b2):
    inp = np.asarray(inp, dtype=np.float32)
    gate_idx = np.asarray(gate_idx)
    gate_score = np.asarray(gate_score, dtype=np.float32)
    w1 = np.asarray(w1, dtype=np.float32)
    b1 = np.asarray(b1, dtype=np.float32)
    w2 = np.asarray(w2, dtype=np.float32)
    b2 = np.asarray(b2, dtype=np.float32)

    routes = _route(gate_idx, gate_score)

    w1t_all = np.ascontiguousarray(w1.transpose(0, 2, 1)).astype(_F16)
    w2t_all = np.ascontiguousarray(w2.transpose(0, 2, 1)).astype(_F16)
    b1r = np.ascontiguousarray(
        b1.reshape(E, KH, 128).transpose(0, 2, 1))  # [E, 128, KH]

    in_maps = [
        {"w1t": np.zeros((EPC, D, H), _F16),
         "w2t": np.zeros((EPC, H, D), _F16),
         "b1": np.zeros((EPC, 128, KH), np.float32),
         "xt0": np.zeros((D, CAPS[0]), _F16),
         "xt1": np.zeros((D, CAPS[1]), _F16)}
        for _ in range(NCORES)
    ]
    for e in range(E):
        toks, wts, otoks, owts, core, slot = routes[e]
        im = in_maps[core]
        im["w1t"][slot] = w1t_all[e]
        im["w2t"][slot] = w2t_all[e]
        im["b1"][slot] = b1r[e]
        if len(toks):
            im[f"xt{slot}"][:, :len(toks)] = inp[toks].T.astype(_F16)

    nc = _get_nc()
    res = run_bass_kernel_spmd(nc, in_maps, list(range(NCORES)))

    # Host combine: weight each expert's output columns by the (summed)
    # gate score and accumulate per token; add the b2 term (folded out of
    # the device kernel). Tokens are unique within an expert, so the
    # fancy-indexed += is safe.
    out = np.einsum("tk,tkd->td", gate_score,
                    b2[np.asarray(gate_idx).astype(np.int64)])
    out = np.ascontiguousarray(out, dtype=np.float32)
    for e in range(E):
        toks, wts, otoks, owts, core, slot = routes[e]
        if len(toks):
            y = res.results[core][f"yt{slot}"][:, :len(toks)].T
            out[toks] += wts[:, None] * y.astype(np.float32)
        if len(otoks):  # exact host fallback for capacity overflow
            hh = inp[otoks] @ w1[e].T + b1[e]
            hh = 0.5 * hh * (1.0 + np.tanh(
                np.sqrt(2.0 / np.pi) * (hh + 0.044715 * hh ** 3)))
            out[otoks] += owts[:, None] * (hh @ w2[e].T)
    return out
